# revision 8
# baseline (speedup 1.0000x reference)
"""Trainium2 Bass kernel for nn_EngramConv: out = silu(dwconv(rmsnorm(x))) + x.

x [4, 4096, 2048] f32. Sharding: 8 cores, core i handles (batch i//2, half i%2)
= 2048 consecutive tokens (+ a 128-token halo tile supplying the 9-token
causal-conv history; zeros at sequence start).

Per-core pipeline (tokens-on-partitions "layout 1" <-> channels-on-partitions
"layout 2"):
  DMA x tile (layout 1, contiguous 8KB rows)
  ACT: square+accum -> sum(x^2); sqrt(+eps, /D); DVE reciprocal -> rstd
  ACT: xn = x * rstd -> bf16
  PE : transpose 128x128 blocks -> xnT (layout 2) via PSUM, DVE/ACT copy to SBUF
  PE : depthwise conv = 4 accumulating matmuls, stationary = diag(w_k) bf16
  ACT: silu(PSUM) -> bf16
  PE : transpose back -> PSUM, DVE residual add (+x) -> f32, DMA out
norm_weight is folded into the conv weights on the host (exact: depthwise conv
commutes with per-channel scaling).
"""

import numpy as np
import ml_dtypes

B, S, D = 4, 4096, 2048
KSZ, DIL = 4, 3
PAD = (KSZ - 1) * DIL  # 9
EPS = 1e-6
N_CORES = 8
TOKC = B * S // N_CORES  # 2048 tokens per core
P = 128
T = 512                   # tokens per main tile
NPT = T // P              # 4 p-tiles per tile
NT = TOKC // T            # 4 main tiles per core
NCH = D // P              # 16 channel chunks

_cache = {}
ACT_NAME = "Silu"  # test_sim swaps to "Sigmoid" (CoreSim lacks a Silu impl)


def _kernel_body(tc, out, x_main, x_halo, wdiag, ident):
    import concourse.bass as bass
    from concourse import mybir
    from contextlib import ExitStack

    nc = tc.nc
    f32 = mybir.dt.float32
    bf16 = mybir.dt.bfloat16
    AF = mybir.ActivationFunctionType

    with ExitStack() as ctx:
        consts = ctx.enter_context(tc.tile_pool(name="consts", bufs=1))
        xpool = ctx.enter_context(tc.tile_pool(name="xpool", bufs=2))
        xnpool = ctx.enter_context(tc.tile_pool(name="xnpool", bufs=2))
        xntp = ctx.enter_context(tc.tile_pool(name="xntp", bufs=1))
        small = ctx.enter_context(tc.tile_pool(name="small", bufs=8))
        silup = ctx.enter_context(tc.tile_pool(name="silup", bufs=2))
        ps_t1 = ctx.enter_context(tc.tile_pool(name="ps_t1", bufs=2, space="PSUM"))
        ps_cv = ctx.enter_context(tc.tile_pool(name="ps_cv", bufs=2, space="PSUM"))
        ps_t2 = ctx.enter_context(tc.tile_pool(name="ps_t2", bufs=2, space="PSUM"))

        # constants
        w_sb = consts.tile([P, NCH, KSZ, P], bf16)
        nc.sync.dma_start(out=w_sb, in_=wdiag)
        id_sb = consts.tile([P, P], bf16)
        nc.sync.dma_start(out=id_sb, in_=ident)
        eps_sb = consts.tile([P, 1], f32)
        nc.vector.memset(eps_sb, EPS)

        # persistent transposed-xn buffers, one per channel chunk:
        # xnt[c][:, j] holds channel chunk c of token (tile_t0 - PAD + j)
        xnt = [
            xntp.tile([P, PAD + T], bf16, tag=f"xnt{c}", name=f"xnt{c}")
            for c in range(NCH)
        ]

        def norm_to_bf16(x_ap, xn_ap):
            """xn_ap = (x_ap * rsqrt(mean(x^2)+eps)) cast bf16; [128, D] tiles."""
            ss = small.tile([P, 1], f32, tag="ss")
            nc.scalar.activation(out=xn_ap, in_=x_ap, func=AF.Square, accum_out=ss)
            rstd = small.tile([P, 1], f32, tag="rstd")
            nc.scalar.activation(
                out=rstd, in_=ss, func=AF.Sqrt, bias=eps_sb, scale=1.0 / D
            )
            nc.vector.reciprocal(out=rstd, in_=rstd)
            nc.scalar.activation(out=xn_ap, in_=x_ap, func=AF.Copy, scale=rstd)

        # ---- halo pre-tile: last PAD tokens feed tile 0's conv taps ----
        hx = consts.tile([P, D], f32, tag="hx")
        nc.sync.dma_start(out=hx, in_=x_halo)
        hxn = consts.tile([P, D], bf16, tag="hxn")
        norm_to_bf16(hx, hxn)
        for c in range(NCH):
            tp = ps_t1.tile([P, T], bf16, tag="t1")
            nc.tensor.transpose(tp[:, 0:P], hxn[:, c * P:(c + 1) * P], id_sb)
            nc.any.tensor_copy(out=xnt[c][:, 0:PAD], in_=tp[:, P - PAD:P])

        # ---- main tiles ----
        for it in range(NT):
            t0 = it * T
            x_t = xpool.tile([P, NPT, D], f32, tag="x")
            nc.sync.dma_start(
                out=x_t, in_=x_main[t0:t0 + T, :].rearrange("(pt p) d -> p pt d", p=P)
            )
            xn_t = xnpool.tile([P, NPT, D], bf16, tag="xn")
            for pt in range(NPT):
                norm_to_bf16(x_t[:, pt], xn_t[:, pt])

            # transpose to layout 2: xnt[c][:, PAD + pt*128 + t] over all pt
            for c in range(NCH):
                tpc = ps_t1.tile([P, T], bf16, tag="t1")
                for pt in range(NPT):
                    nc.tensor.transpose(
                        tpc[:, pt * P:(pt + 1) * P],
                        xn_t[:, pt, c * P:(c + 1) * P],
                        id_sb,
                    )
                nc.any.tensor_copy(out=xnt[c][:, PAD:PAD + T], in_=tpc)

            # depthwise conv: 4 accumulating diag matmuls per chunk
            sl_tiles = {}
            for c in range(NCH):
                cv = ps_cv.tile([P, T], f32, tag="cv")
                for k in range(KSZ):
                    nc.tensor.matmul(
                        cv,
                        w_sb[:, c, k, :],
                        xnt[c][:, 3 * k:3 * k + T],
                        start=(k == 0),
                        stop=(k == KSZ - 1),
                    )
                sl = silup.tile([P, T], bf16, tag=f"sl{c}")
                nc.scalar.activation(out=sl, in_=cv, func=getattr(AF, ACT_NAME))
                sl_tiles[c] = sl

            # transpose back + residual + store
            for pt in range(NPT):
                op = ps_t2.tile([P, D], bf16, tag="t2")
                for c in range(NCH):
                    nc.tensor.transpose(
                        op[:, c * P:(c + 1) * P],
                        sl_tiles[c][:, pt * P:(pt + 1) * P],
                        id_sb,
                    )
                nc.vector.tensor_add(out=x_t[:, pt], in0=x_t[:, pt], in1=op)
            nc.sync.dma_start(
                out=out[t0:t0 + T, :].rearrange("(pt p) d -> p pt d", p=P),
                in_=x_t,
            )

            # slide halo window for next tile
            if it + 1 < NT:
                for c in range(NCH):
                    nc.any.tensor_copy(out=xnt[c][:, 0:PAD], in_=xnt[c][:, T:T + PAD])


def _build():
    if "nc" in _cache:
        return _cache["nc"]
    from concourse import bacc, mybir
    import concourse.tile as tile

    nc = bacc.Bacc(
        "TRN2",
        target_bir_lowering=False,
        debug=False,
        enable_asserts=False,
        num_devices=N_CORES,
    )
    f32 = mybir.dt.float32
    bf16 = mybir.dt.bfloat16
    x_main = nc.dram_tensor("x_main", [TOKC, D], f32, kind="ExternalInput").ap()
    x_halo = nc.dram_tensor("x_halo", [P, D], f32, kind="ExternalInput").ap()
    wdiag = nc.dram_tensor("wdiag", [P, NCH, KSZ, P], bf16, kind="ExternalInput").ap()
    ident = nc.dram_tensor("ident", [P, P], bf16, kind="ExternalInput").ap()
    out = nc.dram_tensor("out", [TOKC, D], f32, kind="ExternalOutput").ap()
    with tile.TileContext(nc) as tc:
        _kernel_body(tc, out, x_main, x_halo, wdiag, ident)
    nc.compile()
    _cache["nc"] = nc
    return nc


def _make_in_maps(x, norm_weight, conv_weight):
    bf = ml_dtypes.bfloat16
    w = (conv_weight[:, 0, :] * norm_weight[:, None]).astype(np.float32)  # [D, 4]
    wdiag = np.zeros((NCH, KSZ, P, P), np.float32)
    for c in range(NCH):
        for k in range(KSZ):
            np.fill_diagonal(wdiag[c, k], w[c * P:(c + 1) * P, k])
    wdiag = np.ascontiguousarray(wdiag.transpose(2, 0, 1, 3)).astype(bf)  # [P,NCH,K,P]
    ident = np.eye(P, dtype=bf)
    zero_halo = np.zeros((P, D), np.float32)
    in_maps = []
    for core in range(N_CORES):
        b, h = core // 2, core % 2
        xm = np.ascontiguousarray(x[b, h * TOKC:(h + 1) * TOKC, :])
        xh = np.ascontiguousarray(x[b, TOKC - P:TOKC, :]) if h == 1 else zero_halo
        in_maps.append({"x_main": xm, "x_halo": xh, "wdiag": wdiag, "ident": ident})
    return in_maps


def _run(inputs, trace=False):
    from concourse import bass_utils

    nc = _build()
    in_maps = _make_in_maps(
        np.asarray(inputs["x"]),
        np.asarray(inputs["norm_weight"]),
        np.asarray(inputs["conv_weight"]),
    )
    kw = {}
    if trace:
        kw = dict(trace=True, trace_cores=list(range(N_CORES)))
    res = bass_utils.run_bass_kernel_spmd(
        nc, in_maps, core_ids=list(range(N_CORES)), **kw
    )
    outs = [res.results[i]["out"] for i in range(N_CORES)]
    full = np.stack(
        [np.concatenate([outs[2 * b], outs[2 * b + 1]], axis=0) for b in range(B)]
    )
    return full, res


def kernel(**inputs):
    full, _ = _run(inputs, trace=False)
    return full
